# revision 1
# baseline (speedup 1.0000x reference)
"""Trainium2 Bass kernel for BinaryMLP:
    h = relu(x @ sign(w1).T + b1); h = relu(h @ sign(w2).T + b2);
    h = relu(h @ sign(w3).T + b3); y = h @ w4.T + b4

Data-parallel over 8 NeuronCores: batch 65536 -> 8192 rows/core, weights
replicated. On-device dataflow is feature-major ("transposed"): activations
live in SBUF as [feature_partition, batch_free] so every layer's contraction
dim (the feature/hidden dim) is the PE partition dim. The host only slices
the batch, transposes/casts for layout, and concatenates the result back.

Compute is bf16 on the tensor engine (binary +-1 weights are exact in bf16;
PSUM accumulates fp32; x is rounded to bf16 host-side — identical numerics
to an on-device cast). Binarization (sign of the latent weights) runs per
k-slice, split across the scalar engine (Sign activation) and the vector
engine (single uint16-bitcast (w & 0x8000) | 0x3F80 tensor_scalar — exact
bf16 +-1), so the sign chain is not serial on one engine during the
DMA-bound prologue.

Prologue is scheduled around the billed window (first framework memset ->
last NEFF instruction): w1 and x-chunk-0/1 stream on the single saturated
sync HWDGE ring in per-k-slice descriptors ordered by PE consumption (the
first fc1 matmul's deps complete after ~170KB instead of ~1.2MB), biases
ride the gpsimd SWDGE ring. The HAM clock gate needs ~3-5us of SUSTAINED
full-array PE power to step 4/8 -> 8/8 (2.4 GHz; DMA clocks lift with it),
and any PE idle gap resets the ramp — so the warm-up matmuls drive the
full 128x128 array and filler dummies are woven into the DMA-paced fc1
stalls to keep the power signal continuous. Chunks are processed in
interleaved pairs (kills layer-boundary stalls; peak PSUM use stays at 8
banks). First and last pairs run at half width (256 cols — the narrowest
width that still hides the ~97ns LDWEIGHTS): the lead pair lets the PE
start on a quarter of the data; the tail pair halves the end-of-kernel
relu/activate/DMA drain that is serial after the last matmul.
"""

import numpy as np

N_CORES = 8
F_IN = 784  # input features: 7 k-tiles of 112
K1 = 112
NK1 = 7
H = 512  # hidden width: 4 k-tiles / m-tiles of 128
NKH = 4
N_OUT = 10
CHUNK = 512  # batch columns per moving-operand chunk
WARMUP = 10  # full-array PE warm-up matmuls (~4us at the pre-ramp half clock)


def build_nc(b_shard: int, num_devices: int = N_CORES, chunk: int = CHUNK):
    """Build + compile the per-core Bass program for a batch shard of
    b_shard columns. Every core runs the identical program."""
    import concourse.bacc as bacc
    import concourse.mybir as mybir
    import concourse.tile as tile

    f32 = mybir.dt.float32
    bf16 = mybir.dt.bfloat16
    ActFn = mybir.ActivationFunctionType
    AluOp = mybir.AluOpType

    # chunk schedule: lead pair and tail pair at half width (fast PE start /
    # short drain), full-width chunks in between. 256 is the floor: narrower
    # matmuls can no longer hide the ~97ns LDWEIGHTS behind the moving pass.
    lead = [chunk // 2, chunk // 2]
    tail = [chunk // 2, chunk // 2]
    assert (b_shard - sum(lead) - sum(tail)) % chunk == 0
    widths = lead + [chunk] * ((b_shard - sum(lead) - sum(tail)) // chunk) + tail
    chunks = []
    off = 0
    for cw in widths:
        chunks.append((off, cw))
        off += cw

    nc = bacc.Bacc(
        "TRN2", target_bir_lowering=False, debug=False, num_devices=num_devices
    )

    xT = nc.dram_tensor("xT", [F_IN, b_shard], bf16, kind="ExternalInput")
    # latent weights ship as bf16: sign() is invariant to bf16 rounding and
    # w4's bf16 cast is the same rounding the kernel would do on-device
    w1T = nc.dram_tensor("w1T", [F_IN, H], bf16, kind="ExternalInput")
    w2T = nc.dram_tensor("w2T", [H, H], bf16, kind="ExternalInput")
    w3T = nc.dram_tensor("w3T", [H, H], bf16, kind="ExternalInput")
    w4T = nc.dram_tensor("w4T", [H, N_OUT], bf16, kind="ExternalInput")
    # biases host-packed to per-partition layout: col 4*l+m = b{l+1}[m*128:(m+1)*128]
    ball = nc.dram_tensor("ball", [128, 12], f32, kind="ExternalInput")
    b4 = nc.dram_tensor("b4", [N_OUT, 1], f32, kind="ExternalInput")
    y = nc.dram_tensor("y", [N_OUT, b_shard], f32, kind="ExternalOutput")

    with tile.TileContext(nc) as tc:
        with (
            tc.tile_pool(name="wconst", bufs=1) as wpool,
            tc.tile_pool(name="wstage", bufs=2) as wstage,
            tc.tile_pool(name="xbf", bufs=4) as xbf_pool,
            tc.tile_pool(name="hbuf", bufs=8) as h_pool,
            tc.tile_pool(name="yout", bufs=4) as y_pool,
            tc.tile_pool(name="psum", bufs=8, space="PSUM") as ps_pool,
        ):
            # PE warm-up seed first, on gpsimd (its queue is empty, so this
            # runs immediately after the framework memsets): the sooner the
            # warm-up starts, the sooner the HAM clock gate ramps to 8/8
            # (2.4 GHz) — which also lifts the DMA clocks. The gate ramps on
            # sustained PE POWER, so the seed must drive the full 128x128
            # array — a tiny stationary tile never triggers it.
            pe_seed = wpool.tile([128, 512], bf16, tag="pe_seed", name="pe_seed")
            nc.gpsimd.memset(pe_seed[:], 1.0)
            # Sign bias: maps w==0 -> +1, matching where(w>=0,1,-1)
            sign_eps = wpool.tile([128, 1], f32, tag="sign_eps", name="sign_eps")
            nc.vector.memset(sign_eps[:], 1e-20)
            # dummy activation: pull the ACT table load off the critical path
            warm = wpool.tile([1, 1], bf16, tag="warm", name="warm")
            nc.scalar.activation(warm[:], sign_eps[0:1, :], ActFn.Sign, bias=0.0)

            # tiny bias loads on the gpsimd SWDGE ring: keeps both HWDGE
            # rings and the scalar engine free for the critical-path w1/x0
            # stream and sign work (biases aren't needed until fc1's relu)
            ballt = wpool.tile([128, 12], f32, tag="ballt", name="ballt")
            nc.gpsimd.dma_start(ballt[:], ball.ap()[:])
            b4t = wpool.tile([N_OUT, 1], f32, tag="b4t", name="b4t")
            nc.gpsimd.dma_start(b4t[:], b4.ap()[:])

            # PE warm-up: full-array dummy matmuls while the prologue DMAs
            # stream. These both bridge the DMA wait AND supply the sustained
            # high-power activity the HAM gate needs (~3us) to step the PE
            # clock 4/8 -> 8/8; filler() dummies are also woven into the
            # DMA-paced fc1 stalls below so the power signal never drops.
            # 512-col dummies: same PE power at half the LDWEIGHTS bandwidth
            # (the 128x128 stationary reload each matmul reads SBUF, which
            # contends with the prologue DMA writes)
            pe_sink = ps_pool.tile([128, 512], f32, tag="ps", name="pe_sink")

            def filler(n, sink=None):
                for _ in range(n):
                    nc.tensor.matmul(
                        (sink if sink is not None else pe_sink)[:],
                        lhsT=pe_seed[:, 0:128], rhs=pe_seed[:],
                        start=True, stop=True,
                    )

            filler(WARMUP)

            # ---- prologue streams ----
            # sync ring: w1 / x0 k-slices interleaved so the first fc1
            # matmul's deps (w1 k0 + x0 k0) land after ~170KB of traffic.
            cwA = chunks[0][1]
            w1f = wstage.tile([K1, NK1, H], bf16, tag="wstage", name="w1bf")
            xb0 = xbf_pool.tile([K1, NK1, cwA], bf16, tag="xb", name="xb0")
            w1src = w1T.ap().rearrange("(a p) n -> p a n", p=K1)
            x0src = xT.ap()[:, 0:cwA].rearrange("(a p) n -> p a n", p=K1)
            for wk, xk in (((0, 1), (0, 2)), ((1, 4), (2, 5)), ((4, 7), (5, 7))):
                nc.sync.dma_start(w1f[:, wk[0]:wk[1], :], w1src[:, wk[0]:wk[1], :])
                nc.sync.dma_start(xb0[:, xk[0]:xk[1], :], x0src[:, xk[0]:xk[1], :])

            # x1 rides the same sync ring AFTER the pair-0 critical stream:
            # the ring is HBM-saturated during the prologue, so packet order
            # must match PE consumption order (a second ring just preempts
            # the critical packets, it doesn't add bandwidth)
            coff1, cw1 = chunks[1]
            xb1 = xbf_pool.tile([K1, NK1, cw1], bf16, tag="xb", name="xb1")
            nc.sync.dma_start(
                xb1[:], xT.ap()[:, coff1:coff1 + cw1].rearrange("(a p) n -> p a n", p=K1)
            )

            # ---- binarization, split across scalar + vector engines ----
            tmp_pool = wstage  # staging for the vector-engine sign pair
            def sign_slice(wb, wf, k, k_size, eng, name):
                if eng == "s":
                    nc.scalar.activation(
                        wb[:, k, :], wf[:, k, :], ActFn.Sign,
                        bias=sign_eps[:k_size, :],
                    )
                else:
                    # single-op sign on the bf16 bit pattern: keep the sign
                    # bit, force exponent/mantissa to 1.0 — so w -> +-1.0
                    # with w == +0 -> +1, matching where(w>=0,1,-1)
                    u16 = mybir.dt.uint16
                    nc.vector.tensor_scalar(
                        wb[:, k, :].bitcast(u16), wf[:, k, :].bitcast(u16),
                        0x8000, 0x3F80,
                        AluOp.bitwise_and, AluOp.bitwise_or,
                    )

            # engine split: k0 on scalar (idle at that point), the rest on
            # vector — the single-op bitwise sign is ~2.5x cheaper than the
            # scalar Sign activation, and the scalar engine is the busier
            # one later (relu m0/m1 + head activations)
            w1b = wpool.tile([K1, NK1, H], bf16, tag="w1b", name="w1b")
            for k, eng in enumerate("svvvvvv"[:NK1]):
                sign_slice(w1b, w1f, k, K1, eng, "w1")

            def load_x(ci, after=None):
                coff, cw = chunks[ci]
                xb = xbf_pool.tile([K1, NK1, cw], bf16, tag="xb", name=f"xb{ci}")
                src = xT.ap()[:, coff:coff + cw].rearrange("(a p) n -> p a n", p=K1)
                dma = nc.sync.dma_start(xb[:], src)
                if after is not None:
                    tile.add_dep_helper(dma.ins, after.ins, sync=True)
                return xb

            def prep_bin(w_dram, n_k, k_size, name, engs, after=None, splits=1):
                wf = wstage.tile([k_size, n_k, H], bf16, tag="wstage", name=f"{name}f")
                src = w_dram.ap().rearrange("(a p) n -> p a n", p=k_size)
                bounds = [round(n_k * s / splits) for s in range(splits + 1)]
                for s in range(splits):
                    k0, k1 = bounds[s], bounds[s + 1]
                    dma = nc.sync.dma_start(wf[:, k0:k1, :], src[:, k0:k1, :])
                    if after is not None:
                        tile.add_dep_helper(dma.ins, after.ins, sync=True)
                wb = wpool.tile([k_size, n_k, H], bf16, tag=name, name=name)
                for k in range(n_k):
                    sign_slice(wb, wf, k, k_size, engs[k], name)
                return wb

            b1t = ballt[:, 0:4]
            b2t = ballt[:, 4:8]
            b3t = ballt[:, 8:12]

            def layer(c, cw, ins_of_k, wtiles, btiles, n_k, name, k_outer=False,
                      fill=0):
                outs = []
                mms = []
                pss = [
                    ps_pool.tile([128, cw], f32, tag="ps", name=f"ps_{name}_{c}_{m}")
                    for m in range(NKH)
                ]
                # k_outer: emit k-round-robin across the 4 psum groups so the
                # PE has ready work as soon as the first k-slices land
                # (prologue only; steady state uses m-outer). fill: dummy
                # matmuls after each k-group, covering the DMA-paced stall so
                # the PE never idles (keeps the HAM clock ramp intact).
                order = (
                    [(m, k) for k in range(n_k) for m in range(NKH)]
                    if k_outer
                    else [(m, k) for m in range(NKH) for k in range(n_k)]
                )
                for m, k in order:
                    mms.append(
                        nc.tensor.matmul(
                            pss[m][:],
                            lhsT=wtiles[:, k, m * 128 : (m + 1) * 128],
                            rhs=ins_of_k(k),
                            start=(k == 0),
                            stop=(k == n_k - 1),
                        )
                    )
                    if fill and m == NKH - 1 and k < n_k - 1:
                        filler(fill)
                for m in range(NKH):
                    ht = h_pool.tile(
                        [128, cw], bf16, tag=f"h{name}", name=f"h{name}_{c}_{m}"
                    )
                    if m < 2:
                        nc.scalar.activation(
                            ht[:], pss[m][:], ActFn.Relu,
                            bias=btiles[:, m : m + 1], scale=1.0,
                        )
                    else:
                        # same math on the otherwise-idle vector engine:
                        # out = max(in + bias, 0) -> halves the relu drain
                        # latency that gates PSUM-bank recycling
                        nc.vector.tensor_scalar(
                            ht[:], pss[m][:], btiles[:, m : m + 1], 0.0,
                            AluOp.add, AluOp.max,
                        )
                    outs.append(ht)
                return mms, outs

            def head(c, coff, cw, h3):
                ps4 = ps_pool.tile([N_OUT, cw], f32, tag="ps", name=f"ps4_{c}")
                for k in range(NKH):
                    nc.tensor.matmul(
                        ps4[:],
                        lhsT=w4c[:, k, :],
                        rhs=h3[k][:],
                        start=(k == 0),
                        stop=(k == NKH - 1),
                    )
                yt = y_pool.tile([N_OUT, cw], f32, tag="yt", name=f"yt_{c}")
                nc.scalar.activation(
                    yt[:], ps4[:], ActFn.Identity, bias=b4t[:], scale=1.0
                )
                nc.sync.dma_start(y.ap()[:, coff : coff + cw], yt[:])

            # ---- pair 0: weight prep interleaved with the layer flow so
            # later loads' packets queue behind what's needed first ----
            cwB = chunks[1][1]
            mmsA, h1A = layer(
                0, cwA, lambda k: xb0[:, k, :], w1b, b1t, NK1, "1", k_outer=True,
                fill=2,
            )
            mmsB, h1B = layer(1, cwB, lambda k: xb1[:, k, :], w1b, b1t, NK1, "1")

            w2b = prep_bin(w2T, NKH, 128, "w2b", "vvvv", after=mmsA[0], splits=2)
            # fc1-B's 4th psum allocation above evicted pe_sink's bank; a new
            # sink bridges the fc1-B -> fc2-A gap while w2 streams + signs
            pe_sink2 = ps_pool.tile([128, 512], f32, tag="ps", name="pe_sink2")
            filler(4, sink=pe_sink2)
            _, h2A = layer(0, cwA, lambda k: h1A[k][:], w2b, b2t, NKH, "2")
            _, h2B = layer(1, cwB, lambda k: h1B[k][:], w2b, b2t, NKH, "2")

            w3b = prep_bin(w3T, NKH, 128, "w3b", "vvvv", after=mmsB[0])
            _, h3A = layer(0, cwA, lambda k: h2A[k][:], w3b, b3t, NKH, "3")
            _, h3B = layer(1, cwB, lambda k: h2B[k][:], w3b, b3t, NKH, "3")

            w4c = wpool.tile([128, NKH, N_OUT], bf16, tag="w4c", name="w4c")
            w4dma = nc.sync.dma_start(
                w4c[:], w4T.ap().rearrange("(a p) n -> p a n", p=128)
            )
            tile.add_dep_helper(w4dma.ins, mmsB[0].ins, sync=True)
            head(0, chunks[0][0], cwA, h3A)
            head(1, chunks[1][0], cwB, h3B)

            # ---- remaining pairs ----
            def do_pair(pair, after=None):
                xbs = [load_x(ci, after=after) for ci in pair]
                hs = []
                for i, ci in enumerate(pair):
                    xb = xbs[i]
                    _, outs = layer(
                        ci, chunks[ci][1], lambda k, xb=xb: xb[:, k, :], w1b, b1t,
                        NK1, "1",
                    )
                    hs.append(outs)
                for name, wb, bt in (("2", w2b, b2t), ("3", w3b, b3t)):
                    hs = [
                        layer(
                            ci, chunks[ci][1], lambda k, h=hs[i]: h[k][:], wb, bt,
                            NKH, name,
                        )[1]
                        for i, ci in enumerate(pair)
                    ]
                for i, ci in enumerate(pair):
                    head(ci, chunks[ci][0], chunks[ci][1], hs[i])

            pairs = [
                list(range(s, min(s + 2, len(chunks))))
                for s in range(2, len(chunks), 2)
            ]
            for pi, pair in enumerate(pairs):
                do_pair(pair, after=mmsA[0] if pi == 0 else None)

    nc.compile()
    return nc


_CACHE = {}


def _get_nc(b_shard: int):
    key = b_shard
    if key not in _CACHE:
        _CACHE[key] = build_nc(b_shard)
    return _CACHE[key]


def make_in_maps(x, w1, b1, w2, b2, w3, b3, w4, b4, n_cores=N_CORES):
    """Host-side layout prep (slicing/transpose/dtype marshalling only)."""
    import ml_dtypes

    B = x.shape[0]
    b_shard = B // n_cores
    xT = np.ascontiguousarray(
        np.asarray(x, dtype=np.float32).T.astype(ml_dtypes.bfloat16)
    )
    ball = np.concatenate(
        [np.asarray(b, np.float32).reshape(NKH, 128).T for b in (b1, b2, b3)], axis=1
    )
    def wprep(w):
        return np.ascontiguousarray(
            np.asarray(w, np.float32).T.astype(ml_dtypes.bfloat16)
        )

    common = {
        "w1T": wprep(w1),
        "w2T": wprep(w2),
        "w3T": wprep(w3),
        "w4T": wprep(w4),
        "ball": np.ascontiguousarray(ball),
        "b4": np.asarray(b4, np.float32).reshape(N_OUT, 1),
    }
    return [
        {"xT": np.ascontiguousarray(xT[:, i * b_shard : (i + 1) * b_shard]), **common}
        for i in range(n_cores)
    ]


def kernel(x, w1, b1, w2, b2, w3, b3, w4, b4):
    from concourse.bass_utils import run_bass_kernel_spmd

    B = x.shape[0]
    b_shard = B // N_CORES
    nc = _get_nc(b_shard)
    in_maps = make_in_maps(x, w1, b1, w2, b2, w3, b3, w4, b4)
    res = run_bass_kernel_spmd(nc, in_maps, core_ids=list(range(N_CORES)))
    yT = np.concatenate([res.results[i]["y"] for i in range(N_CORES)], axis=1)
    return np.ascontiguousarray(yT.T).astype(np.float32)

